# revision 16
# baseline (speedup 1.0000x reference)
"""Trainium2 Bass kernel for nn_NeuralNetGlobalHammer.

Math (per element, on the complex signal z = xr + i*xi):
    t    = xr^2 + xi^2
    mag  = sqrt(t)
    s    = sum_j w2[j] * tanh(w1[j] * mag)        (8-channel MLP)
    p    = s / mag
    xr'  = p * xr ;  xi' = p * xi                 (re-attach phase)
    yr   = conv(xr', wr) - conv(xi', wi)          (32-tap valid FIR along W)
    yi   = conv(xi', wr) + conv(xr', wi)
    out  = SCALE * stack([yr, yi], -1)

Mapping: pure data parallel, core c gets rows [128c, 128c+128) of the
(B*H = 1024, W = 16384) row-major view.  Pointwise runs row-major on
ACT (sqrt, 8x tanh) and DVE (custom fused square-sum / MAC2 tree /
reciprocal / products); the FIR runs on the PE as fp32 matmuls with
the transposed data chunk as the stationary operand and a banded
Toeplitz filter matrix as the moving operand, so conv outputs land
row-major in PSUM and are evacuated with the output scale fused.
"""
import contextlib
import os
import numpy as np

B, H, W = 16, 64, 16384
FL = 32                      # filter taps
WOUT = W - FL + 1            # 16353
ROWS = 128                   # rows per core (B*H / 8)
NCORES = 8
STRIDE = 96                  # FIR outputs per PE window (128 - 32)
NWINC = 14                   # windows per full chunk
CW = NWINC * STRIDE          # 1536 output cols per chunk
X = CW + FL                  # 1568 input cols per chunk tile
GROUPW = 5                   # FIR windows sharing one PSUM bank group
EXPECTED_SI_POWER_DB = -15
SCALE = float(np.sqrt(10.0 ** (EXPECTED_SI_POWER_DB / 10.0)))

_CACHE = {}
_OPS = {}


def _register_custom_ops():
    """Register fused DVE ops (in-process): SQSUM = sq(a)+sq(b),
    MAC2 = a*c0 + b*c1. Shas are computed here and pinned."""
    if _OPS:
        return _OPS
    import concourse.dve_ops as dve_ops
    from concourse.dve_ops import DveOp
    from concourse.dve_spec import Spec, Src0, Src1, C0, C1, sq, lower
    from concourse.dve_spec import _has_src1
    from concourse.dve_uop import DveOpSpec

    defs = [
        ("SQSUM_ANT", Spec(
            body=sq(Src0) + sq(Src1),
            reference=lambda in0, in1, s0, s1, imm2:
                in0.astype(np.float32) ** 2 + in1.astype(np.float32) ** 2)),
        ("MAC2_ANT", Spec(
            body=Src0 * C0 + Src1 * C1,
            reference=lambda in0, in1, s0, s1, imm2:
                in0.astype(np.float32) * s0 + in1.astype(np.float32) * s1)),
    ]
    row = max(dve_ops._SUB_OPCODE_FOR_NAME.values()) + 1
    for name, spec in defs:
        if name in dve_ops._SUB_OPCODE_FOR_NAME:
            _OPS[name] = next(o for o in dve_ops.OPS if o.name == name)
            continue
        assert row < 0x20
        dve_ops._SUB_OPCODE_FOR_NAME[name] = row
        shas = {}
        for ver in ("v3", "v4"):
            try:
                uops = lower(spec, ver=ver)
                shas[ver] = DveOpSpec(name=name, opcode=row, uops=uops,
                                      rd1_en=_has_src1(spec)).sha(ver)
            except Exception:
                pass
        op = DveOp(name, spec, subdim=False, uops_sha=shas)
        dve_ops.OPS.append(op)
        dve_ops.CUSTOM_DVE_SPECS[name] = spec
        _OPS[name] = op
        row += 1
    return _OPS


def _build_program(w1, w2, n_reps=1):
    import concourse.bass as bass
    import concourse.bacc as bacc
    import concourse.mybir as mybir
    import concourse.tile as tile
    from concourse.dve_ops import RECIPROCAL_APPROX_FAST, RECIP_APPROX_FAST_CONSTS

    ops = _register_custom_ops()
    F32 = mybir.dt.float32
    AF = mybir.ActivationFunctionType
    RC = RECIP_APPROX_FAST_CONSTS

    nc = bacc.Bacc("TRN2", target_bir_lowering=False, debug=False,
                   num_devices=NCORES)

    xr_d = nc.declare_dram_parameter("xr", [ROWS, W], F32, isOutput=False)
    xi_d = nc.declare_dram_parameter("xi", [ROWS, W], F32, isOutput=False)
    tr_d = nc.declare_dram_parameter("toe_r", [128, STRIDE], F32, isOutput=False)
    ti_d = nc.declare_dram_parameter("toe_i", [128, STRIDE], F32, isOutput=False)
    tni_d = nc.declare_dram_parameter("toe_ni", [128, STRIDE], F32, isOutput=False)
    eye_d = nc.declare_dram_parameter("eye", [128, 128], F32, isOutput=False)
    out_d = nc.declare_dram_parameter("out", [ROWS, 2 * WOUT], F32, isOutput=True)

    # chunk layout along W
    chunks = []
    for cb in range(0, W, CW):
        lw = min(X, W - cb)                            # valid input width
        nwin = min(NWINC, (WOUT - cb + STRIDE - 1) // STRIDE)
        vout = min(CW, WOUT - cb)                      # valid output cols
        chunks.append((cb, lw, nwin, vout))

    with tile.TileContext(nc) as tc:
        with (
            tc.tile_pool(name="const", bufs=1) as cpool,
            tc.tile_pool(name="io", bufs=3) as iop,
            tc.tile_pool(name="mid", bufs=2) as midp,
            tc.tile_pool(name="magp", bufs=3) as magp,
            tc.tile_pool(name="hb", bufs=2) as hbp,
            tc.tile_pool(name="tmp", bufs=3) as tmpp,
            tc.tile_pool(name="xtb", bufs=3) as xtbp,
            tc.tile_pool(name="stg", bufs=2) as stgp,
            tc.tile_pool(name="tps", bufs=3, space=bass.MemorySpace.PSUM) as tpsp,
            tc.tile_pool(name="ypr", bufs=2, space=bass.MemorySpace.PSUM) as yprp,
            tc.tile_pool(name="ypi", bufs=2, space=bass.MemorySpace.PSUM) as ypip,
        ):
            tr_t = cpool.tile([128, STRIDE], F32, tag="tr")
            ti_t = cpool.tile([128, STRIDE], F32, tag="ti")
            tni_t = cpool.tile([128, STRIDE], F32, tag="tni")
            eye_t = cpool.tile([128, 128], F32, tag="eye")
            nc.sync.dma_start(tr_t[:], tr_d[:])
            nc.sync.dma_start(ti_t[:], ti_d[:])
            nc.sync.dma_start(tni_t[:], tni_d[:])
            nc.sync.dma_start(eye_t[:], eye_d[:])

            rep_ctx = (tc.For_i(0, n_reps, 1) if n_reps > 1
                       else contextlib.nullcontext())
            with rep_ctx:
                _emit_body(nc, chunks, w1, w2,
                           iop, midp, magp, hbp, tmpp, xtbp, stgp, tpsp, yprp, ypip,
                           xr_d, xi_d, out_d, tr_t, ti_t, tni_t, eye_t,
                           F32, AF, ops, RECIPROCAL_APPROX_FAST, RC)

    nc.compile()
    return nc


def _emit_body(nc, chunks, w1, w2,
               iop, midp, magp, hbp, tmpp, xtbp, stgp, tpsp, yprp, ypip,
               xr_d, xi_d, out_d, tr_t, ti_t, tni_t, eye_t,
               F32, AF, ops, RECIP, RC):
    SQSUM = ops["SQSUM_ANT"]
    MAC2 = ops["MAC2_ANT"]

    # Stage 1 (load + mag) is emitted for a PAIR of chunks back-to-back so
    # the two ACT Sqrt ops share one sqrt-table load; the tanh stage then
    # runs with a single tanh-table load per pair (Copy is in every set).
    def stage1(cb, lw):
        xr_t = iop.tile([ROWS, X], F32, tag="xr")
        xi_t = iop.tile([ROWS, X], F32, tag="xi")
        if lw < X:
            nc.vector.memset(xr_t[:, lw:X], 0.0)
            nc.vector.memset(xi_t[:, lw:X], 0.0)
        nc.sync.dma_start(xr_t[:, 0:lw], xr_d[:, cb:cb + lw])
        nc.sync.dma_start(xi_t[:, 0:lw], xi_d[:, cb:cb + lw])

        t_t = midp.tile([ROWS, X], F32, tag="t")
        nc.vector._custom_dve(SQSUM, out=t_t[:], in0=xr_t[:], in1=xi_t[:])
        mag = magp.tile([ROWS, X], F32, tag="mag")
        nc.scalar.activation(mag[:], t_t[:],
                             AF.Square if os.environ.get("K_SQPROBE") else AF.Sqrt)
        inv = magp.tile([ROWS, X], F32, tag="inv")
        nc.vector._custom_dve(RECIP, out=inv[:], in0=mag[:],
                              s0=RC["s0"], s1=RC["s1"], imm2=RC["imm2"])
        return xr_t, xi_t, mag, inv

    pstep = 2 if os.environ.get("K_PAIR", "1") == "1" else 1
    for ci in range(0, len(chunks), pstep):
        pair = chunks[ci:ci + pstep]
        st1 = [stage1(cb, lw) for (cb, lw, _, _) in pair]
        for (cb, lw, nwin, vout), (xr_t, xi_t, mag, inv) in zip(pair, st1):
            _emit_chunk_rest(nc, cb, lw, nwin, vout, xr_t, xi_t, mag, inv,
                             w1, w2, midp, hbp, tmpp, xtbp, stgp, tpsp,
                             yprp, ypip, out_d, tr_t, ti_t, tni_t, eye_t,
                             F32, AF, MAC2)


def _emit_chunk_rest(nc, cb, lw, nwin, vout, xr_t, xi_t, mag, inv,
                     w1, w2, midp, hbp, tmpp, xtbp, stgp, tpsp,
                     yprp, ypip, out_d, tr_t, ti_t, tni_t, eye_t,
                     F32, AF, MAC2):
    if True:
        hs = []
        for j in range(8):
            h_t = hbp.tile([ROWS, X], F32, tag=f"h{j % 2}")
            nc.scalar.activation(h_t[:], mag[:], AF.Tanh, scale=float(w1[j]))
            hs.append(h_t)
        pr = []
        for k in range(4):
            pk = hbp.tile([ROWS, X], F32, tag=f"p{k % 2}")
            nc.vector._custom_dve(MAC2, out=pk[:], in0=hs[2 * k][:],
                                  in1=hs[2 * k + 1][:],
                                  s0=float(w2[2 * k]), s1=float(w2[2 * k + 1]))
            pr.append(pk)
        q0 = tmpp.tile([ROWS, X], F32, tag="tmp")
        nc.vector.tensor_add(q0[:], pr[0][:], pr[1][:])
        q1 = tmpp.tile([ROWS, X], F32, tag="tmp")
        nc.vector.tensor_add(q1[:], pr[2][:], pr[3][:])
        s_t = tmpp.tile([ROWS, X], F32, tag="tmp")
        nc.vector.tensor_add(s_t[:], q0[:], q1[:])
        p_t = tmpp.tile([ROWS, X], F32, tag="tmp")
        nc.vector.tensor_mul(p_t[:], s_t[:], inv[:])
        xp_r = midp.tile([ROWS, X], F32, tag="xp_r")
        nc.vector.tensor_mul(xp_r[:], p_t[:], xr_t[:])
        xp_i = midp.tile([ROWS, X], F32, tag="xp_i")
        nc.vector.tensor_mul(xp_i[:], p_t[:], xi_t[:])
        if lw < X:
            # pad region holds NaN (0/0); zero it so the Toeplitz band
            # zeros actually mask it in the FIR matmuls
            nc.vector.memset(xp_r[:, lw:X], 0.0)
            nc.vector.memset(xp_i[:, lw:X], 0.0)

        stg = stgp.tile([ROWS, 2 * CW], F32, tag="stg")

        # transposes: pairs of windows share one PSUM bank; all evacs on ACT
        xtbs = {}
        for pj in range(0, nwin, 2):
            npair = min(2, nwin - pj)
            tb = tpsp.tile([128, 512], F32, tag="tps")
            for u in range(npair):
                j = pj + u
                o = 256 * u
                nc.tensor.transpose(
                    tb[:, o:o + 128], xp_r[:, STRIDE * j:STRIDE * j + 128],
                    eye_t[:])
                nc.tensor.transpose(
                    tb[:, o + 128:o + 256],
                    xp_i[:, STRIDE * j:STRIDE * j + 128], eye_t[:])
            xtb = xtbp.tile([128, 512], F32, tag="xtb")
            if (pj // 2) % 2 == 0:
                nc.vector.tensor_copy(xtb[:, 0:256 * npair], tb[:, 0:256 * npair])
            else:
                nc.scalar.copy(xtb[:, 0:256 * npair], tb[:, 0:256 * npair])
            for u in range(npair):
                j = pj + u
                o = 256 * u
                xtbs[j] = (xtb[:, o:o + 128], xtb[:, o + 128:o + 256])

        # FIR: groups of GROUPW windows accumulate into one PSUM bank pair
        for g0 in range(0, nwin, GROUPW):
            gw = min(GROUPW, nwin - g0)
            ypr = yprp.tile([128, STRIDE * GROUPW], F32, tag="ypr")
            ypi = ypip.tile([128, STRIDE * GROUPW], F32, tag="ypi")
            for u in range(gw):
                j = g0 + u
                xrT, xiT = xtbs[j]
                sl = slice(STRIDE * u, STRIDE * (u + 1))
                # start=True clears has_written for the whole bank, so only
                # the first matmul into each bank tile sets it; later
                # windows overwrite-on-first-touch then accumulate.
                nc.tensor.matmul(ypr[:, sl], xrT, tr_t[:],
                                 start=(u == 0), stop=False,
                                 skip_group_check=True)
                nc.tensor.matmul(ypi[:, sl], xrT, ti_t[:],
                                 start=(u == 0), stop=False,
                                 skip_group_check=True)
                nc.tensor.matmul(ypr[:, sl], xiT, tni_t[:],
                                 start=False, stop=(u == gw - 1),
                                 skip_group_check=True)
                nc.tensor.matmul(ypi[:, sl], xiT, tr_t[:],
                                 start=False, stop=(u == gw - 1),
                                 skip_group_check=True)
            # evacuate with output scale + re/im interleave (ACT balances DVE)
            so = 192 * g0
            nc.scalar.mul(
                stg[:, so:so + 192 * gw:2], ypr[:, 0:STRIDE * gw], SCALE)
            nc.scalar.mul(
                stg[:, so + 1:so + 192 * gw:2], ypi[:, 0:STRIDE * gw], SCALE)

        nc.sync.dma_start(out_d[:, 2 * cb:2 * (cb + vout)],
                          stg[:, 0:2 * vout])


def _get_program(w1, w2, n_reps=1):
    key = (w1.tobytes(), w2.tobytes(), n_reps)
    if key not in _CACHE:
        _CACHE[key] = _build_program(w1, w2, n_reps)
    return _CACHE[key]


def _toeplitz(taps, sign=1.0):
    t = np.zeros((128, STRIDE), dtype=np.float32)
    for m in range(STRIDE):
        t[m:m + FL, m] = sign * taps
    return t


def kernel(x_real, x_imag, w_nl1, w_nl2, w_lin_real, w_lin_imag,
           _trace=False, _trace_kwargs=None):
    from concourse.bass_utils import run_bass_kernel_spmd

    w1 = np.asarray(w_nl1, dtype=np.float32).reshape(8)
    w2 = np.asarray(w_nl2, dtype=np.float32).reshape(8)
    wr = np.asarray(w_lin_real, dtype=np.float32).reshape(FL)
    wi = np.asarray(w_lin_imag, dtype=np.float32).reshape(FL)

    nc = _get_program(w1, w2)

    xr = np.ascontiguousarray(np.asarray(x_real, np.float32).reshape(B * H, W))
    xi = np.ascontiguousarray(np.asarray(x_imag, np.float32).reshape(B * H, W))
    consts = {
        "toe_r": _toeplitz(wr),
        "toe_i": _toeplitz(wi),
        "toe_ni": _toeplitz(wi, -1.0),
        "eye": np.eye(128, dtype=np.float32),
    }
    in_maps = []
    for c in range(NCORES):
        in_maps.append({
            "xr": np.ascontiguousarray(xr[ROWS * c:ROWS * (c + 1)]),
            "xi": np.ascontiguousarray(xi[ROWS * c:ROWS * (c + 1)]),
            **consts,
        })
    kw = {}
    if _trace:
        kw["trace"] = True
        if _trace_kwargs:
            kw.update(_trace_kwargs)
    res = run_bass_kernel_spmd(nc, in_maps, list(range(NCORES)), **kw)
    out = np.concatenate([res.results[c]["out"].reshape(ROWS, WOUT, 2)
                          for c in range(NCORES)], axis=0)
    out = out.reshape(B, H, WOUT, 1, 2)
    if _trace:
        kernel.last_results = res
    return out


# revision 17
# speedup vs baseline: 1.1263x; 1.1263x over previous
"""Trainium2 Bass kernel for nn_NeuralNetGlobalHammer.

Math (per element, on the complex signal z = xr + i*xi):
    t    = xr^2 + xi^2
    mag  = sqrt(t)
    s    = sum_j w2[j] * tanh(w1[j] * mag)        (8-channel MLP)
    p    = s / mag
    xr'  = p * xr ;  xi' = p * xi                 (re-attach phase)
    yr   = conv(xr', wr) - conv(xi', wi)          (32-tap valid FIR along W)
    yi   = conv(xi', wr) + conv(xr', wi)
    out  = SCALE * stack([yr, yi], -1)

Mapping: pure data parallel, core c gets rows [128c, 128c+128) of the
(B*H = 1024, W = 16384) row-major view.  Pointwise runs row-major on
ACT (sqrt, 8x tanh) and DVE (custom fused square-sum / MAC2 tree /
reciprocal / products); the FIR runs on the PE as fp32 matmuls with
the transposed data chunk as the stationary operand and a banded
Toeplitz filter matrix as the moving operand, so conv outputs land
row-major in PSUM and are evacuated with the output scale fused.
"""
import contextlib
import os
import numpy as np

B, H, W = 16, 64, 16384
FL = 32                      # filter taps
WOUT = W - FL + 1            # 16353
ROWS = 128                   # rows per core (B*H / 8)
NCORES = 8
STRIDE = 96                  # FIR outputs per PE window (128 - 32)
NWINC = 14                   # windows per full chunk
CW = NWINC * STRIDE          # 1536 output cols per chunk
X = CW + FL                  # 1568 input cols per chunk tile
GROUPW = 5                   # FIR windows sharing one PSUM bank group
EXPECTED_SI_POWER_DB = -15
SCALE = float(np.sqrt(10.0 ** (EXPECTED_SI_POWER_DB / 10.0)))

_CACHE = {}
_OPS = {}


def _register_custom_ops():
    """Register fused DVE ops (in-process): SQSUM = sq(a)+sq(b),
    MAC2 = a*c0 + b*c1. Shas are computed here and pinned."""
    if _OPS:
        return _OPS
    import concourse.dve_ops as dve_ops
    from concourse.dve_ops import DveOp
    from concourse.dve_spec import Spec, Src0, Src1, C0, C1, sq, lower
    from concourse.dve_spec import _has_src1
    from concourse.dve_uop import DveOpSpec

    defs = [
        ("SQSUM_ANT", Spec(
            body=sq(Src0) + sq(Src1),
            reference=lambda in0, in1, s0, s1, imm2:
                in0.astype(np.float32) ** 2 + in1.astype(np.float32) ** 2)),
        ("MAC2_ANT", Spec(
            body=Src0 * C0 + Src1 * C1,
            reference=lambda in0, in1, s0, s1, imm2:
                in0.astype(np.float32) * s0 + in1.astype(np.float32) * s1)),
    ]
    row = max(dve_ops._SUB_OPCODE_FOR_NAME.values()) + 1
    for name, spec in defs:
        if name in dve_ops._SUB_OPCODE_FOR_NAME:
            _OPS[name] = next(o for o in dve_ops.OPS if o.name == name)
            continue
        assert row < 0x20
        dve_ops._SUB_OPCODE_FOR_NAME[name] = row
        shas = {}
        for ver in ("v3", "v4"):
            try:
                uops = lower(spec, ver=ver)
                shas[ver] = DveOpSpec(name=name, opcode=row, uops=uops,
                                      rd1_en=_has_src1(spec)).sha(ver)
            except Exception:
                pass
        op = DveOp(name, spec, subdim=False, uops_sha=shas)
        dve_ops.OPS.append(op)
        dve_ops.CUSTOM_DVE_SPECS[name] = spec
        _OPS[name] = op
        row += 1
    return _OPS


def _build_program(w1, w2, n_reps=1):
    import concourse.bass as bass
    import concourse.bacc as bacc
    import concourse.mybir as mybir
    import concourse.tile as tile
    from concourse.dve_ops import RECIPROCAL_APPROX_FAST, RECIP_APPROX_FAST_CONSTS

    ops = _register_custom_ops()
    F32 = mybir.dt.float32
    AF = mybir.ActivationFunctionType
    RC = RECIP_APPROX_FAST_CONSTS

    nc = bacc.Bacc("TRN2", target_bir_lowering=False, debug=False,
                   num_devices=NCORES)

    xr_d = nc.declare_dram_parameter("xr", [ROWS, W], F32, isOutput=False)
    xi_d = nc.declare_dram_parameter("xi", [ROWS, W], F32, isOutput=False)
    tr_d = nc.declare_dram_parameter("toe_r", [128, STRIDE], F32, isOutput=False)
    ti_d = nc.declare_dram_parameter("toe_i", [128, STRIDE], F32, isOutput=False)
    tni_d = nc.declare_dram_parameter("toe_ni", [128, STRIDE], F32, isOutput=False)
    eye_d = nc.declare_dram_parameter("eye", [128, 128], F32, isOutput=False)
    out_d = nc.declare_dram_parameter("out", [ROWS, 2 * WOUT], F32, isOutput=True)

    # chunk layout along W
    chunks = []
    for cb in range(0, W, CW):
        lw = min(X, W - cb)                            # valid input width
        nwin = min(NWINC, (WOUT - cb + STRIDE - 1) // STRIDE)
        vout = min(CW, WOUT - cb)                      # valid output cols
        chunks.append((cb, lw, nwin, vout))

    with tile.TileContext(nc) as tc:
        with (
            tc.tile_pool(name="const", bufs=1) as cpool,
            tc.tile_pool(name="io", bufs=3) as iop,
            tc.tile_pool(name="mid", bufs=2) as midp,
            tc.tile_pool(name="magp", bufs=3) as magp,
            tc.tile_pool(name="hb", bufs=2) as hbp,
            tc.tile_pool(name="tmp", bufs=3) as tmpp,
            tc.tile_pool(name="xtb", bufs=3) as xtbp,
            tc.tile_pool(name="stg", bufs=2) as stgp,
            tc.tile_pool(name="tps", bufs=2, space=bass.MemorySpace.PSUM) as tpsp,
            tc.tile_pool(name="ypr", bufs=2, space=bass.MemorySpace.PSUM) as yprp,
            tc.tile_pool(name="ypi", bufs=2, space=bass.MemorySpace.PSUM) as ypip,
        ):
            tr_t = cpool.tile([128, STRIDE], F32, tag="tr")
            ti_t = cpool.tile([128, STRIDE], F32, tag="ti")
            tni_t = cpool.tile([128, STRIDE], F32, tag="tni")
            eye_t = cpool.tile([128, 128], F32, tag="eye")
            nc.sync.dma_start(tr_t[:], tr_d[:])
            nc.sync.dma_start(ti_t[:], ti_d[:])
            nc.sync.dma_start(tni_t[:], tni_d[:])
            nc.sync.dma_start(eye_t[:], eye_d[:])

            rep_ctx = (tc.For_i(0, n_reps, 1) if n_reps > 1
                       else contextlib.nullcontext())
            with rep_ctx:
                _emit_body(nc, chunks, w1, w2,
                           iop, midp, magp, hbp, tmpp, xtbp, stgp, tpsp, yprp, ypip,
                           xr_d, xi_d, out_d, tr_t, ti_t, tni_t, eye_t,
                           F32, AF, ops, RECIPROCAL_APPROX_FAST, RC)

    nc.compile()
    return nc


def _emit_body(nc, chunks, w1, w2,
               iop, midp, magp, hbp, tmpp, xtbp, stgp, tpsp, yprp, ypip,
               xr_d, xi_d, out_d, tr_t, ti_t, tni_t, eye_t,
               F32, AF, ops, RECIP, RC):
    SQSUM = ops["SQSUM_ANT"]
    MAC2 = ops["MAC2_ANT"]

    # Stage 1 (load + mag) is emitted for a PAIR of chunks back-to-back so
    # the two ACT Sqrt ops share one sqrt-table load; the tanh stage then
    # runs with a single tanh-table load per pair (Copy is in every set).
    def stage1(cb, lw):
        xr_t = iop.tile([ROWS, X], F32, tag="xr")
        xi_t = iop.tile([ROWS, X], F32, tag="xi")
        if lw < X:
            nc.vector.memset(xr_t[:, lw:X], 0.0)
            nc.vector.memset(xi_t[:, lw:X], 0.0)
        nc.sync.dma_start(xr_t[:, 0:lw], xr_d[:, cb:cb + lw])
        nc.sync.dma_start(xi_t[:, 0:lw], xi_d[:, cb:cb + lw])

        t_t = midp.tile([ROWS, X], F32, tag="t")
        nc.vector._custom_dve(SQSUM, out=t_t[:], in0=xr_t[:], in1=xi_t[:])
        mag = magp.tile([ROWS, X], F32, tag="mag")
        nc.scalar.activation(mag[:], t_t[:],
                             AF.Square if os.environ.get("K_SQPROBE") else AF.Sqrt)
        inv = magp.tile([ROWS, X], F32, tag="inv")
        nc.vector._custom_dve(RECIP, out=inv[:], in0=mag[:],
                              s0=RC["s0"], s1=RC["s1"], imm2=RC["imm2"])
        return xr_t, xi_t, mag, inv

    pstep = 2 if os.environ.get("K_PAIR", "1") == "1" else 1
    for ci in range(0, len(chunks), pstep):
        pair = chunks[ci:ci + pstep]
        st1 = [stage1(cb, lw) for (cb, lw, _, _) in pair]
        for (cb, lw, nwin, vout), (xr_t, xi_t, mag, inv) in zip(pair, st1):
            _emit_chunk_rest(nc, cb, lw, nwin, vout, xr_t, xi_t, mag, inv,
                             w1, w2, midp, hbp, tmpp, xtbp, stgp, tpsp,
                             yprp, ypip, out_d, tr_t, ti_t, tni_t, eye_t,
                             F32, AF, MAC2)


def _emit_chunk_rest(nc, cb, lw, nwin, vout, xr_t, xi_t, mag, inv,
                     w1, w2, midp, hbp, tmpp, xtbp, stgp, tpsp,
                     yprp, ypip, out_d, tr_t, ti_t, tni_t, eye_t,
                     F32, AF, MAC2):
    if True:
        hs = []
        for j in range(8):
            h_t = hbp.tile([ROWS, X], F32, tag=f"h{j % 2}")
            nc.scalar.activation(h_t[:], mag[:], AF.Tanh, scale=float(w1[j]))
            hs.append(h_t)
        pr = []
        for k in range(4):
            pk = hbp.tile([ROWS, X], F32, tag=f"p{k % 2}")
            nc.vector._custom_dve(MAC2, out=pk[:], in0=hs[2 * k][:],
                                  in1=hs[2 * k + 1][:],
                                  s0=float(w2[2 * k]), s1=float(w2[2 * k + 1]))
            pr.append(pk)
        q0 = tmpp.tile([ROWS, X], F32, tag="tmp")
        nc.vector.tensor_add(q0[:], pr[0][:], pr[1][:])
        q1 = tmpp.tile([ROWS, X], F32, tag="tmp")
        nc.vector.tensor_add(q1[:], pr[2][:], pr[3][:])
        s_t = tmpp.tile([ROWS, X], F32, tag="tmp")
        nc.vector.tensor_add(s_t[:], q0[:], q1[:])
        p_t = tmpp.tile([ROWS, X], F32, tag="tmp")
        nc.vector.tensor_mul(p_t[:], s_t[:], inv[:])
        xp_r = midp.tile([ROWS, X], F32, tag="xp_r")
        nc.vector.tensor_mul(xp_r[:], p_t[:], xr_t[:])
        xp_i = midp.tile([ROWS, X], F32, tag="xp_i")
        nc.vector.tensor_mul(xp_i[:], p_t[:], xi_t[:])
        if lw < X:
            # pad region holds NaN (0/0); zero it so the Toeplitz band
            # zeros actually mask it in the FIR matmuls
            nc.vector.memset(xp_r[:, lw:X], 0.0)
            nc.vector.memset(xp_i[:, lw:X], 0.0)

        stg = stgp.tile([ROWS, 2 * CW], F32, tag="stg")

        # transposes: 4 windows share one 2-bank PSUM tile; evacs alternate
        # between DVE and ACT to balance engine load
        xtbs = {}
        for pj in range(0, nwin, 4):
            ng = min(4, nwin - pj)
            tb = tpsp.tile([128, 1024], F32, tag="tps")
            for u in range(ng):
                j = pj + u
                o = 256 * u
                nc.tensor.transpose(
                    tb[:, o:o + 128], xp_r[:, STRIDE * j:STRIDE * j + 128],
                    eye_t[:])
                nc.tensor.transpose(
                    tb[:, o + 128:o + 256],
                    xp_i[:, STRIDE * j:STRIDE * j + 128], eye_t[:])
            xtb = xtbp.tile([128, 1024], F32, tag="xtb")
            if (pj // 4) % 2 == 0:
                nc.vector.tensor_copy(xtb[:, 0:256 * ng], tb[:, 0:256 * ng])
            else:
                nc.scalar.copy(xtb[:, 0:256 * ng], tb[:, 0:256 * ng])
            for u in range(ng):
                j = pj + u
                o = 256 * u
                xtbs[j] = (xtb[:, o:o + 128], xtb[:, o + 128:o + 256])

        # FIR: groups of GROUPW windows accumulate into one PSUM bank pair
        for g0 in range(0, nwin, GROUPW):
            gw = min(GROUPW, nwin - g0)
            ypr = yprp.tile([128, STRIDE * GROUPW], F32, tag="ypr")
            ypi = ypip.tile([128, STRIDE * GROUPW], F32, tag="ypi")
            for u in range(gw):
                j = g0 + u
                xrT, xiT = xtbs[j]
                sl = slice(STRIDE * u, STRIDE * (u + 1))
                # start=True clears has_written for the whole bank, so only
                # the first matmul into each bank tile sets it; later
                # windows overwrite-on-first-touch then accumulate.
                nc.tensor.matmul(ypr[:, sl], xrT, tr_t[:],
                                 start=(u == 0), stop=False,
                                 skip_group_check=True)
                nc.tensor.matmul(ypi[:, sl], xrT, ti_t[:],
                                 start=(u == 0), stop=False,
                                 skip_group_check=True)
                nc.tensor.matmul(ypr[:, sl], xiT, tni_t[:],
                                 start=False, stop=(u == gw - 1),
                                 skip_group_check=True)
                nc.tensor.matmul(ypi[:, sl], xiT, tr_t[:],
                                 start=False, stop=(u == gw - 1),
                                 skip_group_check=True)
            # evacuate with output scale + re/im interleave (ACT balances DVE)
            so = 192 * g0
            nc.scalar.mul(
                stg[:, so:so + 192 * gw:2], ypr[:, 0:STRIDE * gw], SCALE)
            nc.scalar.mul(
                stg[:, so + 1:so + 192 * gw:2], ypi[:, 0:STRIDE * gw], SCALE)

        nc.sync.dma_start(out_d[:, 2 * cb:2 * (cb + vout)],
                          stg[:, 0:2 * vout])


def _get_program(w1, w2, n_reps=1):
    key = (w1.tobytes(), w2.tobytes(), n_reps)
    if key not in _CACHE:
        _CACHE[key] = _build_program(w1, w2, n_reps)
    return _CACHE[key]


def _toeplitz(taps, sign=1.0):
    t = np.zeros((128, STRIDE), dtype=np.float32)
    for m in range(STRIDE):
        t[m:m + FL, m] = sign * taps
    return t


def kernel(x_real, x_imag, w_nl1, w_nl2, w_lin_real, w_lin_imag,
           _trace=False, _trace_kwargs=None):
    from concourse.bass_utils import run_bass_kernel_spmd

    w1 = np.asarray(w_nl1, dtype=np.float32).reshape(8)
    w2 = np.asarray(w_nl2, dtype=np.float32).reshape(8)
    wr = np.asarray(w_lin_real, dtype=np.float32).reshape(FL)
    wi = np.asarray(w_lin_imag, dtype=np.float32).reshape(FL)

    nc = _get_program(w1, w2)

    xr = np.ascontiguousarray(np.asarray(x_real, np.float32).reshape(B * H, W))
    xi = np.ascontiguousarray(np.asarray(x_imag, np.float32).reshape(B * H, W))
    consts = {
        "toe_r": _toeplitz(wr),
        "toe_i": _toeplitz(wi),
        "toe_ni": _toeplitz(wi, -1.0),
        "eye": np.eye(128, dtype=np.float32),
    }
    in_maps = []
    for c in range(NCORES):
        in_maps.append({
            "xr": np.ascontiguousarray(xr[ROWS * c:ROWS * (c + 1)]),
            "xi": np.ascontiguousarray(xi[ROWS * c:ROWS * (c + 1)]),
            **consts,
        })
    kw = {}
    if _trace:
        kw["trace"] = True
        if _trace_kwargs:
            kw.update(_trace_kwargs)
    res = run_bass_kernel_spmd(nc, in_maps, list(range(NCORES)), **kw)
    out = np.concatenate([res.results[c]["out"].reshape(ROWS, WOUT, 2)
                          for c in range(NCORES)], axis=0)
    out = out.reshape(B, H, WOUT, 1, 2)
    if _trace:
        kernel.last_results = res
    return out
